# revision 22
# baseline (speedup 1.0000x reference)
"""KAN spline layer (B=16384, IN=512, OUT=1024, cubic B-splines on a uniform
grid, 8 coefficients per (in,out) pair) as a Bass/Tile kernel for 8 Trainium2
NeuronCores.

Strategy
--------
Data-parallel over the batch: each of the 8 cores gets a 2048-row shard of x
and the full (replicated) weights.

Math: with t = x/h - g0/h - 3 in [0.71, 4.29], the 8 cubic B-spline basis
values per (b, i) are 6*B_k(t) = a_k^3 - 4*b_k^3 where
  a_k = min(relu((k+1) - t), relu(t - (k-3)))   (width-4 tent, height 2)
  b_k = min(relu(k - t),     relu(t - (k-2)))   (width-2 tent, height 1)
Each tent-cube is produced by ONE custom DVE instruction (TRI_CUBE_SC:
out = min(relu(s0 - x), relu(x - s1))^3 * imm2, 8 ALU stages).  b_0 and b_7
are identically zero for t in [0.71, 4.29] (x in [0,1)), so k=0/7 planes are
just a^3 and are written to fp8 directly by the DVE op; for k=1..6 the a/b
combine runs on the otherwise-idle GPSIMD engine (two tensor_tensor adds per
chunk for pipelining), writing the planes in fp8e4 in the DoubleRow-
interleaved layout.

Matmuls: the 8 spline planes contract on the PE in fp8 DoubleRow mode
(2 fp8 weights per cell, 256-deep contraction per matmul) against
host-prequantized weights coef*16/6 (error-feedback fp8 quantization along
k); the silu(x) @ scale_base base term stays fp16 for exactness.  Both
accumulate into the same fp32 PSUM; the drain copy scales by 1/16.

All resident weights load via single batched DMAs (one descriptor chain
each) so the per-transfer DGE generation cost does not serialize ahead of
the first batch chunk.
"""

import numpy as np

import concourse.bass as bass
import concourse.mybir as mybir
import concourse.tile as tile
from concourse import bacc
from concourse import dve_ops
from concourse.bass_utils import run_bass_kernel_spmd
from concourse.dve_ops import DveOp
from concourse.dve_spec import C0, C1, C2, Spec, Src0, lower, minn, relu, sq
from concourse.dve_uop import DveOpSpec
from concourse.masks import make_identity

F32 = mybir.dt.float32
F16 = mybir.dt.float16
F8 = mybir.dt.float8e4
ALU = mybir.AluOpType
AFT = mybir.ActivationFunctionType
DR = mybir.MatmulPerfMode.DoubleRow

N_CORES = 8
B_FULL = 16384
BS = B_FULL // N_CORES          # 2048 batch rows per core
IN_DIM = 512
OUT_DIM = 1024
NK = 8                          # spline coefficients per (i, o)
NCH = IN_DIM // 128             # 4 in-dim chunks of 128 partitions
BCH = 256                       # batch columns per chunk iteration
NTB = BCH // 128                # b_tiles per batch chunk (2)
NSUB = NK * NCH                 # 32 contraction subtiles for the spline part
WSCALE = 16.0                   # fp8 weight scale, divided back out on drain


def _make_tricube():
    """Register TRI_CUBE_SC_ANT: out = min(relu(s0-x), relu(x-s1))^3 * imm2."""
    name = "TRI_CUBE_SC_ANT"
    if name in dve_ops._SUB_OPCODE_FOR_NAME:
        return next(op for op in dve_ops.OPS if op.name == name)
    a = minn(relu(C0 - Src0), relu(Src0 - C1))
    body = sq(a) * a * C2

    def ref(in0, in1, s0, s1, imm2):
        x = in0.astype(np.float32)
        av = np.minimum(np.maximum(s0 - x, 0), np.maximum(x - s1, 0))
        return (av * av * av * imm2).astype(np.float32)

    spec = Spec(body=body, reference=ref)
    row = len(dve_ops.OPS) + dve_ops._CUSTOM_DVE_ROW_BASE
    shas = {
        ver: DveOpSpec(name=name, opcode=row, uops=lower(spec, ver=ver),
                       rd1_en=False).sha(ver)
        for ver in ("v3", "v4")
    }
    op = DveOp(name, spec, subdim=False, uops_sha=shas)
    dve_ops.OPS.append(op)
    dve_ops._SUB_OPCODE_FOR_NAME[name] = row
    dve_ops.CUSTOM_DVE_SPECS[name] = op.spec
    return op


TRICUBE = _make_tricube()

K_EDGE = (0, NK - 1)            # b_k identically 0: plane = a^3 straight to fp8
K_MID = tuple(range(1, NK - 1))  # a^3 - 4 b^3 combined per-k on GPSIMD
K_ACT = (1, 6)                  # b-cubes computed on ACT instead of DVE


def kan_body(ctx, tc, y, x, w2, sb, tsc, tbi, bs):
    nc = tc.nc
    nbch = bs // BCH
    NMID = len(K_MID)
    chunks = [(i * BCH, BCH) for i in range(nbch)]

    consts = ctx.enter_context(tc.tile_pool(name="consts", bufs=1))
    xin_pool = ctx.enter_context(tc.tile_pool(name="xin", bufs=3))
    xt_pool = ctx.enter_context(tc.tile_pool(name="xt", bufs=2))
    silu_pool = ctx.enter_context(tc.tile_pool(name="silu", bufs=2))
    cube_pool = ctx.enter_context(tc.tile_pool(name="cubes", bufs=2))
    p8_pool = ctx.enter_context(tc.tile_pool(name="p8", bufs=2))
    yout_pool = ctx.enter_context(tc.tile_pool(name="yout", bufs=2))
    tpsum = ctx.enter_context(tc.tile_pool(name="tpsum", bufs=2, space="PSUM"))
    ypsum = ctx.enter_context(tc.tile_pool(name="ypsum", bufs=3, space="PSUM"))

    def issue_xin(b0, bch):
        ntb = bch // 128
        xin = xin_pool.tile([128, ntb, IN_DIM], F32, tag=f"xin{bch}",
                            name=f"xin{bch}")
        nc.sync.dma_start(
            xin, x[b0:b0 + bch, :].rearrange("(t p) i -> p t i", p=128))
        return xin

    # The first batch chunks' x loads go ahead of the (large) weight DMAs so
    # the DVE pipeline is not stuck behind them on the DMA engines (DMA
    # transfers run in issue order, which follows emission order here).
    xin_pre = {b0: issue_xin(b0, bch) for b0, bch in chunks[:3]}

    # ---- resident constants ---------------------------------------------------
    # small ones first (tp needs tsc/tbi; the first chunks stall if these
    # queue behind the big w2 transfer); w2 in 4 pieces so per-chunk x loads
    # can interleave between them
    tsc_sb = consts.tile([128, NCH, 1], F32)
    nc.sync.dma_start(tsc_sb, tsc.rearrange("(c p) one -> p c one", p=128))
    tbi_sb = consts.tile([128, NCH, 1], F32)
    nc.sync.dma_start(tbi_sb, tbi.rearrange("(c p) one -> p c one", p=128))
    sb_sb = consts.tile([128, NCH, OUT_DIM], F16)
    nc.sync.dma_start(sb_sb, sb.rearrange("(c p) o -> p c o", p=128))
    w2_sb = consts.tile([128, NSUB, OUT_DIM], F8)
    w2v = w2.rearrange("(s p) o -> p s o", p=128)
    for q in range(8):
        sl = slice(q * NSUB // 8, (q + 1) * NSUB // 8)
        nc.sync.dma_start(w2_sb[:, sl, :], w2v[:, sl, :])
    ident = consts.tile([128, 128], F32)
    make_identity(nc, ident)
    bias1 = consts.tile([128, 1], F32)
    nc.vector.memset(bias1, 1.0)
    bias2 = consts.tile([128, 1], F32)
    nc.vector.memset(bias2, 2.0)
    biasd = {}
    for k in K_ACT:
        biasd[k] = consts.tile([128, 1], F32, name=f"biasd{k}", tag=f"biasd{k}")
        nc.vector.memset(biasd[k], float(1 - k))

    for b0, bch in chunks:
        ntb = bch // 128
        hb = 2 if bch == BCH else 1   # half-size tail chunks: single-buffered

        # ---- load + transpose x for this batch chunk ------------------------
        xin = xin_pre.get(b0) or issue_xin(b0, bch)
        xt = xt_pool.tile([128, NCH, bch], F16, tag=f"xt{bch}", name=f"xt{bch}", bufs=hb)
        tp = xt_pool.tile([128, NCH, bch], F16, tag=f"tp{bch}", name=f"tp{bch}", bufs=hb)
        tps = [tpsum.tile([128, NTB, 128], F32, tag=f"tps{c}",
                          name=f"tps{c}", bufs=1)
               for c in range(NCH)]
        for t in range(ntb):
            for c in range(NCH):
                nc.tensor.transpose(tps[c][:, t, :],
                                    xin[:, t, c * 128:(c + 1) * 128], ident)
        for c in range(NCH):
            flat = tps[c][:, :ntb, :].rearrange("p t b -> p (t b)")
            # fp32 PSUM -> fp16 SBUF cast while draining
            nc.scalar.copy(xt[:, c, :], flat)
            # t = x/h - g0/h - 3 per in-dim row
            nc.scalar.activation(tp[:, c, :], flat, AFT.Identity,
                                 bias=tbi_sb[:, c, :],
                                 scale=tsc_sb[:, c, :])

        # ---- silu(x) (base term lhsT), one fused activation -----------------
        silu = silu_pool.tile([128, NCH, bch], F16, tag=f"silu{bch}",
                              name=f"silu{bch}", bufs=hb)
        nc.scalar.activation(silu.rearrange("p c b -> p (c b)"),
                             xt.rearrange("p c b -> p (c b)"), AFT.Silu)

        # ---- spline planes --------------------------------------------------
        # p8[p, s=(k*NCH+c), b]: fp8, DoubleRow-ready (pairs = adjacent s)
        tpf = tp.rearrange("p c b -> p (c b)")
        p8 = p8_pool.tile([128, NSUB, bch], F8, tag=f"p8{bch}",
                          name=f"p8{bch}", bufs=hb)
        a3 = cube_pool.tile([128, NMID, NCH * bch], F16, tag=f"a3{bch}",
                            name=f"a3{bch}", bufs=hb)
        b3 = cube_pool.tile([128, NMID, NCH * bch], F16, tag=f"b3{bch}",
                            name=f"b3{bch}", bufs=hb)
        for k in K_EDGE:
            nc.vector._custom_dve(
                TRICUBE,
                out=p8[:, k * NCH:(k + 1) * NCH, :].rearrange(
                    "p s b -> p (s b)"),
                in0=tpf, s0=float(k + 1), s1=float(k - 3), imm2=1.0)
        for k in K_MID:
            j = k - 1
            nc.vector._custom_dve(TRICUBE, out=a3[:, j, :], in0=tpf,
                                  s0=float(k + 1), s1=float(k - 3), imm2=1.0)
            if k in K_ACT:
                # offload this b-cube to the Activation engine to shorten the
                # DVE critical path: 4*b^3 = Square(2-2d) * Relu(1-d)
                dk = cube_pool.tile([128, NCH * bch], F16,
                                    tag=f"d{k}_{bch}", name=f"d{k}_{bch}", bufs=hb)
                nc.scalar.activation(dk, tpf, AFT.Abs, bias=biasd[k])
                s2 = cube_pool.tile([128, NCH * bch], F16,
                                    tag=f"s2{k}_{bch}", name=f"s2{k}_{bch}", bufs=hb)
                nc.scalar.activation(s2, dk, AFT.Square, bias=bias2,
                                     scale=-2.0)
                rk = cube_pool.tile([128, NCH * bch], F16,
                                    tag=f"r{k}_{bch}", name=f"r{k}_{bch}", bufs=hb)
                nc.scalar.activation(rk, dk, AFT.Relu, bias=bias1, scale=-1.0)
                nc.vector.tensor_tensor(b3[:, j, :], s2, rk, ALU.mult)
                gps_op = ALU.subtract          # p8 = a3 - (+4 b^3)
            else:
                nc.vector._custom_dve(TRICUBE, out=b3[:, j, :], in0=tpf,
                                      s0=float(k), s1=float(k - 2), imm2=-4.0)
                gps_op = ALU.add               # p8 = a3 + (-4 b^3)
            nc.gpsimd.tensor_tensor(
                p8[:, 4 * k:4 * (k + 1), :].rearrange("p s b -> p (s b)"),
                a3[:, j, :], b3[:, j, :], gps_op)

        # ---- matmuls: y = (silu.T @ sb + planes.T @ w2) / WSCALE ------------
        mm_pairs = ([u for k in K_EDGE for u in (2 * k, 2 * k + 1)]
                    + [u for k in K_MID for u in (2 * k, 2 * k + 1)])
        for t in range(ntb):
            yt = yout_pool.tile([128, OUT_DIM], F32)
            for h in range(2):
                ps = ypsum.tile([128, 512], F32)
                o0 = h * 512
                for c in range(NCH):
                    nc.tensor.matmul(ps, silu[:, c, t * 128:(t + 1) * 128],
                                     sb_sb[:, c, o0:o0 + 512],
                                     start=(c == 0), stop=False)
                for n, u in enumerate(mm_pairs):
                    nc.tensor.matmul(
                        ps, p8[:, 2 * u:2 * u + 2, t * 128:(t + 1) * 128],
                        w2_sb[:, 2 * u:2 * u + 2, o0:o0 + 512],
                        start=False, stop=(n == len(mm_pairs) - 1),
                        perf_mode=DR)
                nc.scalar.activation(yt[:, o0:o0 + 512], ps, AFT.Identity,
                                     scale=1.0 / WSCALE)
            nc.sync.dma_start(y[b0 + t * 128: b0 + (t + 1) * 128, :], yt)


def build_nc(bs=BS):
    from contextlib import ExitStack

    nc = bacc.Bacc("TRN2", target_bir_lowering=False, debug=False)
    x = nc.dram_tensor("x", [bs, IN_DIM], F32, kind="ExternalInput").ap()
    w2 = nc.dram_tensor("w2", [NSUB * 128, OUT_DIM], F8,
                        kind="ExternalInput").ap()
    sb = nc.dram_tensor("sb", [IN_DIM, OUT_DIM], F16, kind="ExternalInput").ap()
    tsc = nc.dram_tensor("tsc", [IN_DIM, 1], F32, kind="ExternalInput").ap()
    tbi = nc.dram_tensor("tbi", [IN_DIM, 1], F32, kind="ExternalInput").ap()
    y = nc.dram_tensor("y", [bs, OUT_DIM], F32, kind="ExternalOutput").ap()
    with tile.TileContext(nc) as tc:
        with ExitStack() as ctx:
            kan_body(ctx, tc, y, x, w2, sb, tsc, tbi, bs)
    nc.compile()
    return nc


def host_prep(grid, coef, scale_base):
    grid = np.asarray(grid, dtype=np.float32)
    coef = np.asarray(coef, dtype=np.float32)
    f8np = mybir.dt.np(F8)
    g0 = grid[:, 0]
    h = (grid[:, -1] - grid[:, 0]) / np.float32(grid.shape[1] - 1)
    tsc = (1.0 / h).astype(np.float32).reshape(-1, 1)
    tbi = (-g0 / h - 3.0).astype(np.float32).reshape(-1, 1)
    # error-feedback fp8 quantization of coef*WSCALE/6 along k (adjacent
    # B-spline planes overlap, so pushing the rounding residual into the
    # next k partially cancels in the contraction)
    w = coef * np.float32(WSCALE / 6.0)          # [I, O, K]
    wq = np.empty_like(w)
    r = np.zeros(w.shape[:2], np.float32)
    for k in range(NK):
        v = w[:, :, k] + r
        q = v.astype(f8np).astype(np.float32)
        r = v - q
        wq[:, :, k] = q
    # dram layout: row (k*NCH + c)*128 + p  ->  coef[c*128+p, :, k]
    w2d = np.empty((NSUB * 128, OUT_DIM), f8np)
    for k in range(NK):
        for c in range(NCH):
            s = k * NCH + c
            w2d[s * 128:(s + 1) * 128] = wq[c * 128:(c + 1) * 128, :, k]
    sbv = np.ascontiguousarray(
        np.asarray(scale_base, np.float32) * np.float32(WSCALE)
    ).astype(np.float16)
    return w2d, sbv, tsc, tbi


_NC_CACHE = {}


def get_nc(bs=BS):
    if bs not in _NC_CACHE:
        _NC_CACHE[bs] = build_nc(bs)
    return _NC_CACHE[bs]


def kernel(x, grid, coef, scale_base):
    x = np.ascontiguousarray(np.asarray(x, dtype=np.float32))
    w2, sbv, tsc, tbi = host_prep(grid, coef, scale_base)
    nc = get_nc(BS)
    in_maps = [
        {"x": x[c * BS:(c + 1) * BS], "w2": w2, "sb": sbv,
         "tsc": tsc, "tbi": tbi}
        for c in range(N_CORES)
    ]
    res = run_bass_kernel_spmd(nc, in_maps, core_ids=list(range(N_CORES)))
    return np.concatenate([res.results[c]["y"] for c in range(N_CORES)], axis=0)


# revision 23
# speedup vs baseline: 1.0148x; 1.0148x over previous
"""KAN spline layer (B=16384, IN=512, OUT=1024, cubic B-splines on a uniform
grid, 8 coefficients per (in,out) pair) as a Bass/Tile kernel for 8 Trainium2
NeuronCores.

Strategy
--------
Data-parallel over the batch: each of the 8 cores gets a 2048-row shard of x
and the full (replicated) weights.

Math: with t = x/h - g0/h - 3 in [0.71, 4.29], the 8 cubic B-spline basis
values per (b, i) are 6*B_k(t) = a_k^3 - 4*b_k^3 where
  a_k = min(relu((k+1) - t), relu(t - (k-3)))   (width-4 tent, height 2)
  b_k = min(relu(k - t),     relu(t - (k-2)))   (width-2 tent, height 1)
Each tent-cube is produced by ONE custom DVE instruction (TRI_CUBE_SC:
out = min(relu(s0 - x), relu(x - s1))^3 * imm2, 8 ALU stages).  b_0 and b_7
are identically zero for t in [0.71, 4.29] (x in [0,1)), so k=0/7 planes are
just a^3 and are written to fp8 directly by the DVE op; for k=1..6 the a/b
combine runs on the otherwise-idle GPSIMD engine (two tensor_tensor adds per
chunk for pipelining), writing the planes in fp8e4 in the DoubleRow-
interleaved layout.

Matmuls: the 8 spline planes contract on the PE in fp8 DoubleRow mode
(2 fp8 weights per cell, 256-deep contraction per matmul) against
host-prequantized weights coef*16/6 (error-feedback fp8 quantization along
k); the silu(x) @ scale_base base term stays fp16 for exactness.  Both
accumulate into the same fp32 PSUM; the drain copy scales by 1/16.

All resident weights load via single batched DMAs (one descriptor chain
each) so the per-transfer DGE generation cost does not serialize ahead of
the first batch chunk.
"""

import numpy as np

import concourse.bass as bass
import concourse.mybir as mybir
import concourse.tile as tile
from concourse import bacc
from concourse import dve_ops
from concourse.bass_utils import run_bass_kernel_spmd
from concourse.dve_ops import DveOp
from concourse.dve_spec import C0, C1, C2, Spec, Src0, lower, minn, relu, sq
from concourse.dve_uop import DveOpSpec
from concourse.masks import make_identity

F32 = mybir.dt.float32
F16 = mybir.dt.float16
F8 = mybir.dt.float8e4
ALU = mybir.AluOpType
AFT = mybir.ActivationFunctionType
DR = mybir.MatmulPerfMode.DoubleRow

N_CORES = 8
B_FULL = 16384
BS = B_FULL // N_CORES          # 2048 batch rows per core
IN_DIM = 512
OUT_DIM = 1024
NK = 8                          # spline coefficients per (i, o)
NCH = IN_DIM // 128             # 4 in-dim chunks of 128 partitions
BCH = 256                       # batch columns per chunk iteration
NTB = BCH // 128                # b_tiles per batch chunk (2)
NSUB = NK * NCH                 # 32 contraction subtiles for the spline part
WSCALE = 16.0                   # fp8 weight scale, divided back out on drain


def _make_tricube():
    """Register TRI_CUBE_SC_ANT: out = min(relu(s0-x), relu(x-s1))^3 * imm2."""
    name = "TRI_CUBE_SC_ANT"
    if name in dve_ops._SUB_OPCODE_FOR_NAME:
        return next(op for op in dve_ops.OPS if op.name == name)
    a = minn(relu(C0 - Src0), relu(Src0 - C1))
    body = sq(a) * a * C2

    def ref(in0, in1, s0, s1, imm2):
        x = in0.astype(np.float32)
        av = np.minimum(np.maximum(s0 - x, 0), np.maximum(x - s1, 0))
        return (av * av * av * imm2).astype(np.float32)

    spec = Spec(body=body, reference=ref)
    row = len(dve_ops.OPS) + dve_ops._CUSTOM_DVE_ROW_BASE
    shas = {
        ver: DveOpSpec(name=name, opcode=row, uops=lower(spec, ver=ver),
                       rd1_en=False).sha(ver)
        for ver in ("v3", "v4")
    }
    op = DveOp(name, spec, subdim=False, uops_sha=shas)
    dve_ops.OPS.append(op)
    dve_ops._SUB_OPCODE_FOR_NAME[name] = row
    dve_ops.CUSTOM_DVE_SPECS[name] = op.spec
    return op


TRICUBE = _make_tricube()

K_EDGE = (0, NK - 1)            # b_k identically 0: plane = a^3 straight to fp8
K_MID = tuple(range(1, NK - 1))  # a^3 - 4 b^3 combined per-k on GPSIMD
K_ACT = (1, 6)                  # b-cubes computed on ACT instead of DVE


def kan_body(ctx, tc, y, x, w2, sb, tsc, tbi, bs):
    nc = tc.nc
    nbch = bs // BCH
    NMID = len(K_MID)
    chunks = [(i * BCH, BCH) for i in range(nbch)]

    consts = ctx.enter_context(tc.tile_pool(name="consts", bufs=1))
    xin_pool = ctx.enter_context(tc.tile_pool(name="xin", bufs=3))
    xt_pool = ctx.enter_context(tc.tile_pool(name="xt", bufs=2))
    silu_pool = ctx.enter_context(tc.tile_pool(name="silu", bufs=2))
    cube_pool = ctx.enter_context(tc.tile_pool(name="cubes", bufs=2))
    p8_pool = ctx.enter_context(tc.tile_pool(name="p8", bufs=2))
    yout_pool = ctx.enter_context(tc.tile_pool(name="yout", bufs=2))
    tpsum = ctx.enter_context(tc.tile_pool(name="tpsum", bufs=2, space="PSUM"))
    ypsum = ctx.enter_context(tc.tile_pool(name="ypsum", bufs=4, space="PSUM"))

    def issue_xin(b0, bch):
        ntb = bch // 128
        xin = xin_pool.tile([128, ntb, IN_DIM], F32, tag=f"xin{bch}",
                            name=f"xin{bch}")
        nc.sync.dma_start(
            xin, x[b0:b0 + bch, :].rearrange("(t p) i -> p t i", p=128))
        return xin

    # The first batch chunks' x loads go ahead of the (large) weight DMAs so
    # the DVE pipeline is not stuck behind them on the DMA engines (DMA
    # transfers run in issue order, which follows emission order here).
    xin_pre = {b0: issue_xin(b0, bch) for b0, bch in chunks[:3]}

    # ---- resident constants ---------------------------------------------------
    # small ones first (tp needs tsc/tbi; the first chunks stall if these
    # queue behind the big w2 transfer); w2 in 4 pieces so per-chunk x loads
    # can interleave between them
    tsc_sb = consts.tile([128, NCH, 1], F32)
    nc.sync.dma_start(tsc_sb, tsc.rearrange("(c p) one -> p c one", p=128))
    tbi_sb = consts.tile([128, NCH, 1], F32)
    nc.sync.dma_start(tbi_sb, tbi.rearrange("(c p) one -> p c one", p=128))
    sb_sb = consts.tile([128, NCH, OUT_DIM], F16)
    nc.sync.dma_start(sb_sb, sb.rearrange("(c p) o -> p c o", p=128))
    w2_sb = consts.tile([128, NSUB, OUT_DIM], F8)
    w2v = w2.rearrange("(s p) o -> p s o", p=128)
    for q in range(8):
        sl = slice(q * NSUB // 8, (q + 1) * NSUB // 8)
        nc.sync.dma_start(w2_sb[:, sl, :], w2v[:, sl, :])
    ident = consts.tile([128, 128], F32)
    make_identity(nc, ident)
    bias1 = consts.tile([128, 1], F32)
    nc.vector.memset(bias1, 1.0)
    bias2 = consts.tile([128, 1], F32)
    nc.vector.memset(bias2, 2.0)
    biasd = {}
    for k in K_ACT:
        biasd[k] = consts.tile([128, 1], F32, name=f"biasd{k}", tag=f"biasd{k}")
        nc.vector.memset(biasd[k], float(1 - k))

    for b0, bch in chunks:
        ntb = bch // 128
        hb = 2 if bch == BCH else 1   # half-size tail chunks: single-buffered

        # ---- load + transpose x for this batch chunk ------------------------
        xin = xin_pre.get(b0) or issue_xin(b0, bch)
        xt = xt_pool.tile([128, NCH, bch], F16, tag=f"xt{bch}", name=f"xt{bch}", bufs=hb)
        tp = xt_pool.tile([128, NCH, bch], F16, tag=f"tp{bch}", name=f"tp{bch}", bufs=hb)
        tps = [tpsum.tile([128, NTB, 128], F32, tag=f"tps{c}",
                          name=f"tps{c}", bufs=1)
               for c in range(NCH)]
        for t in range(ntb):
            for c in range(NCH):
                nc.tensor.transpose(tps[c][:, t, :],
                                    xin[:, t, c * 128:(c + 1) * 128], ident)
        for c in range(NCH):
            flat = tps[c][:, :ntb, :].rearrange("p t b -> p (t b)")
            # fp32 PSUM -> fp16 SBUF cast while draining
            nc.scalar.copy(xt[:, c, :], flat)
            # t = x/h - g0/h - 3 per in-dim row
            nc.scalar.activation(tp[:, c, :], flat, AFT.Identity,
                                 bias=tbi_sb[:, c, :],
                                 scale=tsc_sb[:, c, :])

        # ---- silu(x) (base term lhsT), one fused activation -----------------
        silu = silu_pool.tile([128, NCH, bch], F16, tag=f"silu{bch}",
                              name=f"silu{bch}", bufs=hb)
        nc.scalar.activation(silu.rearrange("p c b -> p (c b)"),
                             xt.rearrange("p c b -> p (c b)"), AFT.Silu)

        # ---- spline planes --------------------------------------------------
        # p8[p, s=(k*NCH+c), b]: fp8, DoubleRow-ready (pairs = adjacent s)
        tpf = tp.rearrange("p c b -> p (c b)")
        p8 = p8_pool.tile([128, NSUB, bch], F8, tag=f"p8{bch}",
                          name=f"p8{bch}", bufs=hb)
        a3 = cube_pool.tile([128, NMID, NCH * bch], F16, tag=f"a3{bch}",
                            name=f"a3{bch}", bufs=hb)
        b3 = cube_pool.tile([128, NMID, NCH * bch], F16, tag=f"b3{bch}",
                            name=f"b3{bch}", bufs=hb)
        for k in K_EDGE:
            nc.vector._custom_dve(
                TRICUBE,
                out=p8[:, k * NCH:(k + 1) * NCH, :].rearrange(
                    "p s b -> p (s b)"),
                in0=tpf, s0=float(k + 1), s1=float(k - 3), imm2=1.0)
        for k in K_MID:
            j = k - 1
            nc.vector._custom_dve(TRICUBE, out=a3[:, j, :], in0=tpf,
                                  s0=float(k + 1), s1=float(k - 3), imm2=1.0)
            if k in K_ACT:
                # offload this b-cube to the Activation engine to shorten the
                # DVE critical path: 4*b^3 = Square(2-2d) * Relu(1-d)
                dk = cube_pool.tile([128, NCH * bch], F16,
                                    tag=f"d{k}_{bch}", name=f"d{k}_{bch}", bufs=hb)
                nc.scalar.activation(dk, tpf, AFT.Abs, bias=biasd[k])
                s2 = cube_pool.tile([128, NCH * bch], F16,
                                    tag=f"s2{k}_{bch}", name=f"s2{k}_{bch}", bufs=hb)
                nc.scalar.activation(s2, dk, AFT.Square, bias=bias2,
                                     scale=-2.0)
                rk = cube_pool.tile([128, NCH * bch], F16,
                                    tag=f"r{k}_{bch}", name=f"r{k}_{bch}", bufs=hb)
                nc.scalar.activation(rk, dk, AFT.Relu, bias=bias1, scale=-1.0)
                nc.vector.tensor_tensor(b3[:, j, :], s2, rk, ALU.mult)
                gps_op = ALU.subtract          # p8 = a3 - (+4 b^3)
            else:
                nc.vector._custom_dve(TRICUBE, out=b3[:, j, :], in0=tpf,
                                      s0=float(k), s1=float(k - 2), imm2=-4.0)
                gps_op = ALU.add               # p8 = a3 + (-4 b^3)
            nc.gpsimd.tensor_tensor(
                p8[:, 4 * k:4 * (k + 1), :].rearrange("p s b -> p (s b)"),
                a3[:, j, :], b3[:, j, :], gps_op)

        # ---- matmuls: y = (silu.T @ sb + planes.T @ w2) / WSCALE ------------
        mm_pairs = ([u for k in K_EDGE for u in (2 * k, 2 * k + 1)]
                    + [u for k in K_MID for u in (2 * k, 2 * k + 1)])
        for t in range(ntb):
            for h in range(2):
                yt = yout_pool.tile([128, 512], F32, tag=f"yt{h}",
                                    name=f"yt{h}")
                ps = ypsum.tile([128, 512], F32)
                o0 = h * 512
                for c in range(NCH):
                    nc.tensor.matmul(ps, silu[:, c, t * 128:(t + 1) * 128],
                                     sb_sb[:, c, o0:o0 + 512],
                                     start=(c == 0), stop=False)
                for n, u in enumerate(mm_pairs):
                    nc.tensor.matmul(
                        ps, p8[:, 2 * u:2 * u + 2, t * 128:(t + 1) * 128],
                        w2_sb[:, 2 * u:2 * u + 2, o0:o0 + 512],
                        start=False, stop=(n == len(mm_pairs) - 1),
                        perf_mode=DR)
                nc.scalar.activation(yt, ps, AFT.Identity,
                                     scale=1.0 / WSCALE)
                # per-half writeback so the last drain overlaps the other
                # half's matmuls instead of serializing at end of kernel
                nc.sync.dma_start(
                    y[b0 + t * 128: b0 + (t + 1) * 128, o0:o0 + 512], yt)


def build_nc(bs=BS):
    from contextlib import ExitStack

    nc = bacc.Bacc("TRN2", target_bir_lowering=False, debug=False)
    x = nc.dram_tensor("x", [bs, IN_DIM], F32, kind="ExternalInput").ap()
    w2 = nc.dram_tensor("w2", [NSUB * 128, OUT_DIM], F8,
                        kind="ExternalInput").ap()
    sb = nc.dram_tensor("sb", [IN_DIM, OUT_DIM], F16, kind="ExternalInput").ap()
    tsc = nc.dram_tensor("tsc", [IN_DIM, 1], F32, kind="ExternalInput").ap()
    tbi = nc.dram_tensor("tbi", [IN_DIM, 1], F32, kind="ExternalInput").ap()
    y = nc.dram_tensor("y", [bs, OUT_DIM], F32, kind="ExternalOutput").ap()
    with tile.TileContext(nc) as tc:
        with ExitStack() as ctx:
            kan_body(ctx, tc, y, x, w2, sb, tsc, tbi, bs)
    nc.compile()
    return nc


def host_prep(grid, coef, scale_base):
    grid = np.asarray(grid, dtype=np.float32)
    coef = np.asarray(coef, dtype=np.float32)
    f8np = mybir.dt.np(F8)
    g0 = grid[:, 0]
    h = (grid[:, -1] - grid[:, 0]) / np.float32(grid.shape[1] - 1)
    tsc = (1.0 / h).astype(np.float32).reshape(-1, 1)
    tbi = (-g0 / h - 3.0).astype(np.float32).reshape(-1, 1)
    # error-feedback fp8 quantization of coef*WSCALE/6 along k (adjacent
    # B-spline planes overlap, so pushing the rounding residual into the
    # next k partially cancels in the contraction)
    w = coef * np.float32(WSCALE / 6.0)          # [I, O, K]
    wq = np.empty_like(w)
    r = np.zeros(w.shape[:2], np.float32)
    for k in range(NK):
        v = w[:, :, k] + r
        q = v.astype(f8np).astype(np.float32)
        r = v - q
        wq[:, :, k] = q
    # dram layout: row (k*NCH + c)*128 + p  ->  coef[c*128+p, :, k]
    w2d = np.empty((NSUB * 128, OUT_DIM), f8np)
    for k in range(NK):
        for c in range(NCH):
            s = k * NCH + c
            w2d[s * 128:(s + 1) * 128] = wq[c * 128:(c + 1) * 128, :, k]
    sbv = np.ascontiguousarray(
        np.asarray(scale_base, np.float32) * np.float32(WSCALE)
    ).astype(np.float16)
    return w2d, sbv, tsc, tbi


_NC_CACHE = {}


def get_nc(bs=BS):
    if bs not in _NC_CACHE:
        _NC_CACHE[bs] = build_nc(bs)
    return _NC_CACHE[bs]


def kernel(x, grid, coef, scale_base):
    x = np.ascontiguousarray(np.asarray(x, dtype=np.float32))
    w2, sbv, tsc, tbi = host_prep(grid, coef, scale_base)
    nc = get_nc(BS)
    in_maps = [
        {"x": x[c * BS:(c + 1) * BS], "w2": w2, "sb": sbv,
         "tsc": tsc, "tbi": tbi}
        for c in range(N_CORES)
    ]
    res = run_bass_kernel_spmd(nc, in_maps, core_ids=list(range(N_CORES)))
    return np.concatenate([res.results[c]["y"] for c in range(N_CORES)], axis=0)
